# revision 1
# baseline (speedup 1.0000x reference)
"""Trainium2 Bass kernel for the BetaBernoulliMixture problem.

Math reformulation (no gammaln needed):
  post_mixweight = 1 / (1 + exp(d + c0)),  c0 = log((1-w)/w), and the
  betaln-difference d = clog2 - clog1 telescopes into a per-row prefix
  sum along T:
    d[t]   = sum_{tau<t} ( ln(num2[tau]) - ln(den2[tau]) )
    num2   = num * (ab1 + tau),   den2 = den * (ab2 + tau)
    num    = obs ? a2 : b2,       den  = obs ? a1 : b1
    a_i    = alpha_i + s_prev,    b_i  = beta_i + f_prev
    ab_i   = alpha_i + beta_i
  with s_prev/f_prev the shifted cumulative success/failure counts.

Mapping (B=4096 rows split 512/core across 8 cores; rows on SBUF
partitions, T on the free dim, F=2048 t-chunks):
  DVE : a1-scan (tensor_tensor_scan), d-scan (2-input scan fuses the
        lnum2-lden2 subtract for free), den select (copy_predicated
        in place on b1), b1 = (iota+ab1+t0) - a1 (scalar_tensor_tensor),
        num2 = (iota+ab1+t0)*num (stt), sel_delta = obs*dd + dbe (ts).
  ACT : a2/b2 bias adds, w2t = iota+ab2+t0, both Ln's, and the sigmoid
        chain exp -> ln1p -> exp (all funcs in the single table set
        natural_log_exp_and_others; other sets are masked out so the
        table is loaded exactly once).
  GPS : num = den + sel_delta, den2 = den * w2t (the only two
        full-size ops GPSIMD can do at acceptable cost).
Per-row constants are packed host-side into one [RPC, NCONST] tensor.
"""

import numpy as np

B, T = 4096, 8192
NCORES = 8
RPC = B // NCORES        # rows per core = 512
P = 128                  # SBUF partitions
RC_N = RPC // P          # row chunks per core = 4
F = 2048                 # t-chunk width
TC_N = T // F            # t chunks = 4
# al1, be1, dal, dbe, dd, then (ab1+t0, ab2+t0) per chunk
NCONST = 5 + 2 * TC_N

_PROGRAM_CACHE = {}


def _patch_act_tables():
    """Restrict activation-table selection to natural_log_exp_and_others
    (keeps dict order so act_func_set_id indices stay valid)."""
    import concourse.bacc as bacc_mod
    import concourse.hw_specs as hw_specs
    if getattr(bacc_mod, "_act_tables_patched", False):
        return
    orig = hw_specs.get_activation_tables

    def filtered(arch):
        full = orig(arch)
        return {
            name: (funcs if name == "natural_log_exp_and_others" else set())
            for name, funcs in full.items()
        }

    bacc_mod.get_activation_tables = filtered
    bacc_mod._act_tables_patched = True


def _build_program(c0: float):
    import concourse.bacc as bacc
    import concourse.mybir as mybir
    from concourse.tile import TileContext

    _patch_act_tables()

    f32 = mybir.dt.float32
    Alu = mybir.AluOpType
    Act = mybir.ActivationFunctionType

    nc = bacc.Bacc()
    obs_d = nc.dram_tensor("obs", [RPC, T], f32, kind="ExternalInput")
    rcst_d = nc.dram_tensor("rowconst", [RPC, NCONST], f32, kind="ExternalInput")
    a1_o = nc.dram_tensor("a1_out", [RPC, T], f32, kind="ExternalOutput")
    b1_o = nc.dram_tensor("b1_out", [RPC, T], f32, kind="ExternalOutput")
    a2_o = nc.dram_tensor("a2_out", [RPC, T], f32, kind="ExternalOutput")
    b2_o = nc.dram_tensor("b2_out", [RPC, T], f32, kind="ExternalOutput")
    pm_o = nc.dram_tensor("post_out", [RPC, T], f32, kind="ExternalOutput")

    with TileContext(nc) as tc:
        with (
            tc.tile_pool(name="consts", bufs=1) as cpool,
            tc.tile_pool(name="rows", bufs=2) as rpool,
            tc.tile_pool(name="work", bufs=2) as wpool,
        ):
            iota_t = cpool.tile([P, F], f32, tag="iota")
            nc.gpsimd.iota(
                iota_t[:], pattern=[[1, F]], base=0, channel_multiplier=0,
                allow_small_or_imprecise_dtypes=True,
            )
            c0_t = cpool.tile([P, 1], f32, tag="c0")
            nc.vector.memset(c0_t[:], c0)

            for rc in range(RC_N):
                r0 = rc * P
                rows_t = rpool.tile([P, NCONST], f32, tag="rows")
                nc.sync.dma_start(rows_t[:], rcst_d[r0:r0 + P, :])
                al1 = rows_t[:, 0:1]
                be1 = rows_t[:, 1:2]
                dal = rows_t[:, 2:3]
                dbe = rows_t[:, 3:4]
                dd = rows_t[:, 4:5]

                prev_a1 = prev_d = None
                for tci in range(TC_N):
                    t0 = tci * F
                    ab1t = rows_t[:, 5 + 2 * tci:6 + 2 * tci]
                    ab2t = rows_t[:, 6 + 2 * tci:7 + 2 * tci]
                    obs_t = wpool.tile([P, F], f32, tag="obs")
                    nc.sync.dma_start(obs_t[:], obs_d[r0:r0 + P, t0:t0 + F])

                    # a1: exclusive scan of obs with init alpha1 (col 0 = init)
                    a1_t = wpool.tile([P, F + 1], f32, tag="a1")
                    a1_init = al1 if tci == 0 else prev_a1[:, F:F + 1]
                    nc.vector.tensor_copy(a1_t[:, 0:1], a1_init)
                    nc.vector.tensor_tensor_scan(
                        a1_t[:, 1:F + 1], obs_t[:], obs_t[:], a1_init,
                        Alu.add, Alu.bypass,
                    )
                    a1_v = a1_t[:, 0:F]

                    # b1 = (t + ab1) - a1  (one stt, no scan, no chain)
                    b1_t = wpool.tile([P, F], f32, tag="b1")
                    nc.vector.scalar_tensor_tensor(
                        b1_t[:], iota_t[:], ab1t, a1_v, Alu.add, Alu.subtract)

                    # outputs a2/b2 via ACT bias adds
                    a2_t = wpool.tile([P, F], f32, tag="a2")
                    b2_t = wpool.tile([P, F], f32, tag="b2")
                    nc.scalar.activation(a2_t[:], a1_v, Act.Identity, bias=dal)
                    nc.scalar.activation(b2_t[:], b1_t[:], Act.Identity, bias=dbe)

                    # split output DMAs across issuers: the sync HWDGE ring
                    # alone serializes ~96 MB of FIFO traffic; GPSIMD's SWDGE
                    # queues carry two of the outputs in parallel
                    # (measured: 459.9us vs 464.4us all-sync)
                    nc.gpsimd.dma_start(a1_o[r0:r0 + P, t0:t0 + F], a1_v)
                    nc.gpsimd.dma_start(b1_o[r0:r0 + P, t0:t0 + F], b1_t[:])
                    nc.sync.dma_start(a2_o[r0:r0 + P, t0:t0 + F], a2_t[:])
                    nc.sync.dma_start(b2_o[r0:r0 + P, t0:t0 + F], b2_t[:])

                    # den = obs ? a1 : b1  (in place on b1 after its DMA + b2)
                    obs_mask = obs_t[:].bitcast(mybir.dt.uint32)
                    nc.vector.copy_predicated(b1_t[:], obs_mask, a1_v)

                    # num = den + (obs ? dal : dbe); overwrite b2 after DMA
                    seld_t = wpool.tile([P, F], f32, tag="seld")
                    nc.vector.tensor_scalar(
                        seld_t[:], obs_t[:], dd, dbe, Alu.mult, Alu.add)
                    nc.gpsimd.tensor_tensor(b2_t[:], b1_t[:], seld_t[:], Alu.add)

                    # num2 = (t + ab1) * num   (stt on DVE)
                    num2_t = wpool.tile([P, F], f32, tag="num2")
                    nc.vector.scalar_tensor_tensor(
                        num2_t[:], iota_t[:], ab1t, b2_t[:], Alu.add, Alu.mult)
                    # den2 = den * (t + ab2)   (w2t from ACT, mult on GPSIMD)
                    w2t_t = wpool.tile([P, F], f32, tag="w2t")
                    nc.scalar.activation(w2t_t[:], iota_t[:], Act.Identity, bias=ab2t)
                    den2_t = wpool.tile([P, F], f32, tag="den2")
                    nc.gpsimd.tensor_tensor(den2_t[:], b1_t[:], w2t_t[:], Alu.mult)

                    # logs in place
                    nc.scalar.activation(num2_t[:], num2_t[:], Act.Ln)
                    nc.scalar.activation(den2_t[:], den2_t[:], Act.Ln)

                    # d: state = (lnum2 + state) - lden2, chained
                    d_t = wpool.tile([P, F + 1], f32, tag="d")
                    if tci == 0:
                        nc.vector.memset(d_t[:, 0:1], 0.0)
                        d_init = 0.0
                    else:
                        d_init = prev_d[:, F:F + 1]
                        nc.vector.tensor_copy(d_t[:, 0:1], d_init)
                    nc.vector.tensor_tensor_scan(
                        d_t[:, 1:F + 1], num2_t[:], den2_t[:], d_init,
                        Alu.add, Alu.subtract,
                    )

                    # post = exp(-ln(1+exp(d+c0)))
                    post_t = wpool.tile([P, F], f32, tag="post")
                    nc.scalar.activation(post_t[:], d_t[:, 0:F], Act.Exp, bias=c0_t[:, 0:1])
                    nc.scalar.activation(post_t[:], post_t[:], Act.Ln, bias=1.0)
                    nc.scalar.activation(post_t[:], post_t[:], Act.Exp, scale=-1.0)
                    nc.sync.dma_start(pm_o[r0:r0 + P, t0:t0 + F], post_t[:])

                    prev_a1, prev_d = a1_t, d_t
    nc.finalize()
    return nc


def _pack_rowconst(alpha1, beta1, alpha2, beta2):
    """[B, NCONST] fp32: al1, be1, dal, dbe, dd, then (ab1+t0, ab2+t0)."""
    a1 = alpha1.astype(np.float32)
    b1 = beta1.astype(np.float32)
    a2 = alpha2.astype(np.float32)
    b2 = beta2.astype(np.float32)
    dal = a2 - a1
    dbe = b2 - b1
    cols = [a1, b1, dal, dbe, dal - dbe]
    ab1 = a1 + b1
    ab2 = a2 + b2
    for tci in range(TC_N):
        t0 = np.float32(tci * F)
        cols.append(ab1 + t0)
        cols.append(ab2 + t0)
    return np.ascontiguousarray(np.stack(cols, axis=1), dtype=np.float32)


def kernel(obs_seq, alpha1, beta1, alpha2, beta2, mixweight):
    from concourse.bass_utils import run_bass_kernel_spmd

    w = float(np.float32(mixweight))
    c0 = float(np.float32(np.log((1.0 - w) / w)))
    key = c0
    if key not in _PROGRAM_CACHE:
        _PROGRAM_CACHE[key] = _build_program(c0)
    nc = _PROGRAM_CACHE[key]

    obs_seq = np.ascontiguousarray(obs_seq, dtype=np.float32)
    rowconst = _pack_rowconst(
        np.asarray(alpha1), np.asarray(beta1),
        np.asarray(alpha2), np.asarray(beta2),
    )
    in_maps = []
    for c in range(NCORES):
        r0 = c * RPC
        in_maps.append({
            "obs": obs_seq[r0:r0 + RPC],
            "rowconst": rowconst[r0:r0 + RPC],
        })
    res = run_bass_kernel_spmd(nc, in_maps, core_ids=list(range(NCORES)))
    out = np.empty((5, B, T), np.float32)
    names = ["a1_out", "b1_out", "a2_out", "b2_out", "post_out"]
    for c in range(NCORES):
        r0 = c * RPC
        for k, name in enumerate(names):
            out[k, r0:r0 + RPC] = res.results[c][name]
    return out



# revision 20
# speedup vs baseline: 1.1438x; 1.1438x over previous
"""Trainium2 Bass kernel for the BetaBernoulliMixture problem.

Math reformulation (no gammaln needed):
  post_mixweight = sigmoid(-(d + c0)),  c0 = log((1-w)/w), and the
  betaln-difference d = clog2 - clog1 telescopes into a per-row prefix
  sum along T:
    d[t]   = sum_{tau<t} ( ln(num2[tau]) - ln(den2[tau]) )
    num2   = num * (ab1 + tau),   den2 = den * (ab2 + tau)
    num    = obs ? a2 : b2,       den  = obs ? a1 : b1
    a_i    = alpha_i + s_prev,    b_i  = beta_i + f_prev
    ab_i   = alpha_i + beta_i
  with s_prev/f_prev the shifted cumulative success/failure counts.

v2 engine mapping (B=4096 rows split 512/core across 8 cores; rows on
SBUF partitions, T on the free dim, F=2048 t-chunks, rc-inner order):
  DVE : a1-scan, d-scan, den select (copy_predicated in place on the
        PSUM b1), small carry copies.
  PE  : u = ab1 + t into PSUM bank A (affine matmul from [ab1; ones]
        weights x [ones; iota] rows), b1 = u - a1 into PSUM bank B
        (same affine then accumulate (-I) @ a1).  fp32 matmuls.
  GPS : numP = obs*dd + den (stt), num2 = (numP + dbe) * u (stt),
        den2 = (iota + ab2t) * den (stt).
  ACT : a2 = a1 + dal (bf16 out), b2 = b1 + dbe (bf16), b1 -> bf16,
        Ln(num2), Ln(den2), post = Sigmoid(-d - c0) (bf16).  Uses two
        activation tables (ln/exp + sigmoid); posts are batched per
        t-chunk across the 4 row-chunks so the table switch cost is
        2 loads per t-chunk.
Outputs: a1 fp32, b1/a2/b2/post bf16 (host upcasts to fp32).
"""

import numpy as np

B, T = 4096, 8192
NCORES = 8
RPC = B // NCORES        # rows per core = 512
P = 128                  # SBUF partitions
RC_N = RPC // P          # row chunks per core = 4
F = 2048                 # t-chunk width
TC_N = T // F            # t chunks = 4
# al1, dal, dbe, dd, then (ab1+t0, ab2+t0) per chunk
NCONST = 4 + 2 * TC_N

_PROGRAM_CACHE = {}


def _patch_act_tables():
    """Restrict activation-table selection to natural_log_exp_and_others
    + sigmoid_and_others (keeps dict order so act_func_set_id stays valid)."""
    import concourse.bacc as bacc_mod
    import concourse.hw_specs as hw_specs
    if getattr(bacc_mod, "_act_tables_patched", False):
        return
    orig = hw_specs.get_activation_tables
    keep = {"natural_log_exp_and_others", "sigmoid_and_others"}

    def filtered(arch):
        full = orig(arch)
        return {
            name: (funcs if name in keep else set())
            for name, funcs in full.items()
        }

    bacc_mod.get_activation_tables = filtered
    bacc_mod._act_tables_patched = True


def _build_program(c0: float):
    import concourse.bacc as bacc
    import concourse.mybir as mybir
    from concourse.tile import TileContext

    _patch_act_tables()

    f32 = mybir.dt.float32
    bf16 = mybir.dt.bfloat16
    Alu = mybir.AluOpType
    Act = mybir.ActivationFunctionType

    nc = bacc.Bacc()
    obs_d = nc.dram_tensor("obs", [RPC, T], f32, kind="ExternalInput")
    rcst_d = nc.dram_tensor("rowconst", [RPC, NCONST], f32, kind="ExternalInput")
    wcst_d = nc.dram_tensor("wconst", [2, RPC], f32, kind="ExternalInput")
    negi_d = nc.dram_tensor("negI", [P, P], f32, kind="ExternalInput")
    rmat_d = nc.dram_tensor("rmat", [2, T], f32, kind="ExternalInput")
    a1_o = nc.dram_tensor("a1_out", [RPC, T], f32, kind="ExternalOutput")
    b1_o = nc.dram_tensor("b1_out", [RPC, T], f32, kind="ExternalOutput")
    a2_o = nc.dram_tensor("a2_out", [RPC, T], bf16, kind="ExternalOutput")
    b2_o = nc.dram_tensor("b2_out", [RPC, T], bf16, kind="ExternalOutput")
    pm_o = nc.dram_tensor("post_out", [RPC, T], bf16, kind="ExternalOutput")

    with TileContext(nc) as tc:
        with (
            tc.tile_pool(name="consts", bufs=1) as cpool,
            tc.tile_pool(name="a1p", bufs=2) as a1pool,
            tc.tile_pool(name="dp", bufs=RC_N + 1) as dpool,
            tc.tile_pool(name="work", bufs=2) as wpool,
            tc.tile_pool(name="psum", bufs=2, space="PSUM") as ppool,
        ):
            iota_t = cpool.tile([P, F], f32, tag="iota")
            nc.gpsimd.iota(
                iota_t[:], pattern=[[1, F]], base=0, channel_multiplier=0,
                allow_small_or_imprecise_dtypes=True,
            )

            c0n_t = cpool.tile([P, 1], f32, tag="c0n")
            nc.vector.memset(c0n_t[:], -c0)
            negi_t = cpool.tile([P, P], f32, tag="negI")
            nc.sync.dma_start(negi_t[:], negi_d[:, :])
            wcst_t = cpool.tile([2, RPC], f32, tag="wconst")
            nc.sync.dma_start(wcst_t[:], wcst_d[:, :])

            rows = []
            for rc in range(RC_N):
                r0 = rc * P
                rt = cpool.tile([P, NCONST], f32, tag=f"rows{rc}")
                nc.sync.dma_start(rt[:], rcst_d[r0:r0 + P, :])
                rows.append(rt)
            carry_a1 = [cpool.tile([P, 1], f32, tag=f"ca1_{rc}", name=f"ca1_{rc}")
                        for rc in range(RC_N)]
            carry_d = [cpool.tile([P, 1], f32, tag=f"cd_{rc}", name=f"cd_{rc}")
                       for rc in range(RC_N)]

            for tci in range(TC_N):
                t0 = tci * F
                # R rows for the affine matmuls: row0 = ones, row1 = iota
                r_t = wpool.tile([2, F], f32, tag="R")
                nc.sync.dma_start(r_t[:], rmat_d[:, t0:t0 + F])
                d_tiles = []
                for rc in range(RC_N):
                    r0 = rc * P
                    rt = rows[rc]
                    al1 = rt[:, 0:1]
                    dal = rt[:, 1:2]
                    dbe = rt[:, 2:3]
                    dd = rt[:, 3:4]
                    ab1t = rt[:, 4 + 2 * tci:5 + 2 * tci]
                    ab2t = rt[:, 5 + 2 * tci:6 + 2 * tci]

                    obs_t = wpool.tile([P, F], f32, tag="obs")
                    nc.sync.dma_start(obs_t[:], obs_d[r0:r0 + P, t0:t0 + F])

                    # a1: exclusive scan of obs with init alpha1
                    a1_t = a1pool.tile([P, F + 1], f32, tag="a1")
                    a1_init = al1 if tci == 0 else carry_a1[rc][:]
                    nc.vector.tensor_copy(a1_t[:, 0:1], a1_init)
                    nc.vector.tensor_tensor_scan(
                        a1_t[:, 1:F + 1], obs_t[:], obs_t[:], a1_init,
                        Alu.add, Alu.bypass,
                    )
                    nc.vector.tensor_copy(carry_a1[rc][:], a1_t[:, F:F + 1])
                    a1_v = a1_t[:, 0:F]

                    # PE: b1 = (ab1 + t) - a1 into PSUM
                    b_ps = ppool.tile([P, F], f32, tag="b1")
                    wsl = wcst_t[:, r0:r0 + P]
                    for q in range(F // 512):
                        sl = slice(q * 512, (q + 1) * 512)
                        rsl = r_t[:, q * 512:(q + 1) * 512]
                        nc.tensor.matmul(
                            b_ps[:, sl], wsl, rsl, start=True, stop=False)
                        nc.tensor.matmul(
                            b_ps[:, sl], negi_t[:], a1_v[:, sl],
                            start=False, stop=True)

                    # b1 -> SBUF fp32 (also the b1 output), then outputs
                    b1_t = wpool.tile([P, F], f32, tag="b1sb")
                    nc.scalar.activation(b1_t[:], b_ps[:], Act.Identity)
                    a2_t = wpool.tile([P, F], bf16, tag="a2")
                    nc.scalar.activation(a2_t[:], a1_v, Act.Identity, bias=dal)
                    b2_t = wpool.tile([P, F], bf16, tag="b2")
                    nc.scalar.activation(b2_t[:], b1_t[:], Act.Identity, bias=dbe)
                    nc.sync.dma_start(a1_o[r0:r0 + P, t0:t0 + F], a1_v)
                    nc.sync.dma_start(b1_o[r0:r0 + P, t0:t0 + F], b1_t[:])
                    nc.sync.dma_start(a2_o[r0:r0 + P, t0:t0 + F], a2_t[:])
                    nc.sync.dma_start(b2_o[r0:r0 + P, t0:t0 + F], b2_t[:])

                    # den = obs ? a1 : b1  (in place in SBUF)
                    obs_mask = obs_t[:].bitcast(mybir.dt.uint32)
                    nc.vector.copy_predicated(b1_t[:], obs_mask, a1_v)

                    # seld = obs*dd + dbe (ACT); num = den + seld (GPS)
                    # num2 = (iota + ab1t) * num (DVE stt)
                    # w2t = iota + ab2t (ACT); den2 = den * w2t (GPS)
                    seld_t = wpool.tile([P, F], f32, tag="seld")
                    nc.scalar.activation(
                        seld_t[:], obs_t[:], Act.Identity, bias=dbe, scale=dd)
                    num2_t = wpool.tile([P, F], f32, tag="num2")
                    nc.gpsimd.tensor_tensor(
                        num2_t[:], b1_t[:], seld_t[:], Alu.add)
                    nc.vector.scalar_tensor_tensor(
                        num2_t[:], iota_t[:], ab1t, num2_t[:], Alu.add, Alu.mult)
                    den2_t = wpool.tile([P, F], f32, tag="den2")
                    nc.scalar.activation(
                        den2_t[:], iota_t[:], Act.Identity, bias=ab2t)
                    nc.gpsimd.tensor_tensor(
                        den2_t[:], b1_t[:], den2_t[:], Alu.mult)

                    # logs in place
                    nc.scalar.activation(num2_t[:], num2_t[:], Act.Ln)
                    nc.scalar.activation(den2_t[:], den2_t[:], Act.Ln)

                    # d: state = (lnum2 + state) - lden2, chained
                    d_t = dpool.tile([P, F + 1], f32, tag="d")
                    if tci == 0:
                        nc.vector.memset(d_t[:, 0:1], 0.0)
                        d_init = 0.0
                    else:
                        d_init = carry_d[rc][:]
                        nc.vector.tensor_copy(d_t[:, 0:1], d_init)
                    nc.vector.tensor_tensor_scan(
                        d_t[:, 1:F + 1], num2_t[:], den2_t[:], d_init,
                        Alu.add, Alu.subtract,
                    )
                    nc.vector.tensor_copy(carry_d[rc][:], d_t[:, F:F + 1])
                    d_tiles.append(d_t)

                # post = sigmoid(-(d + c0)), batched so the act-table
                # switches cost 2 loads per t-chunk
                for rc in range(RC_N):
                    r0 = rc * P
                    post_t = wpool.tile([P, F], bf16, tag="post")
                    nc.scalar.activation(
                        post_t[:], d_tiles[rc][:, 0:F], Act.Sigmoid,
                        bias=c0n_t[:], scale=-1.0)
                    nc.sync.dma_start(pm_o[r0:r0 + P, t0:t0 + F], post_t[:])
    nc.finalize()
    return nc


def _pack_inputs(alpha1, beta1, alpha2, beta2):
    """rowconst [B, NCONST]: al1, dal, dbe, dd, then ab2+t0 per chunk.
    wconst [2, B]: row0 = ab1, row1 = ones."""
    a1 = alpha1.astype(np.float32)
    b1 = beta1.astype(np.float32)
    a2 = alpha2.astype(np.float32)
    b2 = beta2.astype(np.float32)
    dal = a2 - a1
    dbe = b2 - b1
    cols = [a1, dal, dbe, dal - dbe]
    ab1 = a1 + b1
    ab2 = a2 + b2
    for tci in range(TC_N):
        cols.append(ab1 + np.float32(tci * F))
        cols.append(ab2 + np.float32(tci * F))
    rowconst = np.ascontiguousarray(np.stack(cols, axis=1), dtype=np.float32)
    wconst = np.ascontiguousarray(
        np.stack([a1 + b1, np.ones_like(a1)], axis=0), dtype=np.float32)
    return rowconst, wconst


def make_in_maps(obs_seq, alpha1, beta1, alpha2, beta2):
    obs_seq = np.ascontiguousarray(obs_seq, dtype=np.float32)
    rowconst, wconst = _pack_inputs(
        np.asarray(alpha1), np.asarray(beta1),
        np.asarray(alpha2), np.asarray(beta2),
    )
    negI = np.ascontiguousarray(-np.eye(P, dtype=np.float32))
    rmat = np.ascontiguousarray(
        np.stack([np.ones(T, np.float32),
                  np.arange(T, dtype=np.float32)], axis=0))
    in_maps = []
    for c in range(NCORES):
        r0 = c * RPC
        in_maps.append({
            "obs": obs_seq[r0:r0 + RPC],
            "rowconst": rowconst[r0:r0 + RPC],
            "wconst": np.ascontiguousarray(wconst[:, r0:r0 + RPC]),
            "negI": negI,
            "rmat": rmat,
        })
    return in_maps


def kernel(obs_seq, alpha1, beta1, alpha2, beta2, mixweight):
    from concourse.bass_utils import run_bass_kernel_spmd

    w = float(np.float32(mixweight))
    c0 = float(np.float32(np.log((1.0 - w) / w)))
    key = c0
    if key not in _PROGRAM_CACHE:
        _PROGRAM_CACHE[key] = _build_program(c0)
    nc = _PROGRAM_CACHE[key]

    in_maps = make_in_maps(obs_seq, alpha1, beta1, alpha2, beta2)
    res = run_bass_kernel_spmd(nc, in_maps, core_ids=list(range(NCORES)))
    out = np.empty((5, B, T), np.float32)
    names = ["a1_out", "b1_out", "a2_out", "b2_out", "post_out"]
    for c in range(NCORES):
        r0 = c * RPC
        for k, name in enumerate(names):
            out[k, r0:r0 + RPC] = np.asarray(res.results[c][name]).astype(np.float32)
    return out


# revision 26
# speedup vs baseline: 1.2234x; 1.0695x over previous
"""Trainium2 Bass kernel for the BetaBernoulliMixture problem.

Math reformulation (no gammaln needed):
  post_mixweight = sigmoid(-(d + c0)),  c0 = log((1-w)/w), and the
  betaln-difference d = clog2 - clog1 telescopes into a per-row prefix
  sum along T:
    d[t]   = sum_{tau<t} ( ln(num2[tau]) - ln(den2[tau]) )
    num2   = num * (ab1 + tau),   den2 = den * (ab2 + tau)
    num    = obs ? a2 : b2,       den  = obs ? a1 : b1
    a_i    = alpha_i + s_prev,    b_i  = beta_i + f_prev
    ab_i   = alpha_i + beta_i
  with s_prev/f_prev the shifted cumulative success/failure counts.

v2 engine mapping (B=4096 rows split 512/core across 8 cores; rows on
SBUF partitions, T on the free dim, F=2048 t-chunks, rc-inner order):
  DVE : a1-scan, d-scan, den select (copy_predicated in place on the
        PSUM b1), small carry copies.
  PE  : u = ab1 + t into PSUM bank A (affine matmul from [ab1; ones]
        weights x [ones; iota] rows), b1 = u - a1 into PSUM bank B
        (same affine then accumulate (-I) @ a1).  fp32 matmuls.
  GPS : numP = obs*dd + den (stt), num2 = (numP + dbe) * u (stt),
        den2 = (iota + ab2t) * den (stt).
  ACT : a2 = a1 + dal (bf16 out), b2 = b1 + dbe (bf16), b1 -> bf16,
        Ln(num2), Ln(den2), post = Sigmoid(-d - c0) (bf16).  Uses two
        activation tables (ln/exp + sigmoid); posts are batched per
        t-chunk across the 4 row-chunks so the table switch cost is
        2 loads per t-chunk.
Outputs: a1 fp32, b1/a2/b2/post bf16 (host upcasts to fp32).
"""

import numpy as np

B, T = 4096, 8192
NCORES = 8
RPC = B // NCORES        # rows per core = 512
P = 128                  # SBUF partitions
RC_N = RPC // P          # row chunks per core = 4
F = 2048                 # t-chunk width
TC_N = T // F            # t chunks = 4
# al1, dal, dbe, dd, then (ab1+t0, ab2+t0) per chunk
NCONST = 4 + 2 * TC_N

_PROGRAM_CACHE = {}


def _patch_act_tables():
    """Restrict activation-table selection to natural_log_exp_and_others
    + sigmoid_and_others (keeps dict order so act_func_set_id stays valid)."""
    import concourse.bacc as bacc_mod
    import concourse.hw_specs as hw_specs
    if getattr(bacc_mod, "_act_tables_patched", False):
        return
    orig = hw_specs.get_activation_tables
    keep = {"natural_log_exp_and_others", "sigmoid_and_others"}

    def filtered(arch):
        full = orig(arch)
        return {
            name: (funcs if name in keep else set())
            for name, funcs in full.items()
        }

    bacc_mod.get_activation_tables = filtered
    bacc_mod._act_tables_patched = True


def _build_program(c0: float):
    import concourse.bacc as bacc
    import concourse.mybir as mybir
    from concourse.tile import TileContext

    _patch_act_tables()

    f32 = mybir.dt.float32
    bf16 = mybir.dt.bfloat16
    Alu = mybir.AluOpType
    Act = mybir.ActivationFunctionType

    nc = bacc.Bacc()
    obs_d = nc.dram_tensor("obs", [RPC, T], f32, kind="ExternalInput")
    rcst_d = nc.dram_tensor("rowconst", [RPC, NCONST], f32, kind="ExternalInput")
    wcst_d = nc.dram_tensor("wconst", [4, RPC], bf16, kind="ExternalInput")
    negi_d = nc.dram_tensor("negI", [P, P], f32, kind="ExternalInput")
    rmat_d = nc.dram_tensor("rmat", [4, T], bf16, kind="ExternalInput")
    a1_o = nc.dram_tensor("a1_out", [RPC, T], f32, kind="ExternalOutput")
    b1_o = nc.dram_tensor("b1_out", [RPC, T], f32, kind="ExternalOutput")
    a2_o = nc.dram_tensor("a2_out", [RPC, T], bf16, kind="ExternalOutput")
    b2_o = nc.dram_tensor("b2_out", [RPC, T], bf16, kind="ExternalOutput")
    pm_o = nc.dram_tensor("post_out", [RPC, T], bf16, kind="ExternalOutput")

    with TileContext(nc) as tc:
        with (
            tc.tile_pool(name="consts", bufs=1) as cpool,
            tc.tile_pool(name="a1p", bufs=2) as a1pool,
            tc.tile_pool(name="dp", bufs=RC_N + 1) as dpool,
            tc.tile_pool(name="work", bufs=2) as wpool,
            tc.tile_pool(name="psum", bufs=2, space="PSUM") as ppool,
        ):
            iota_t = cpool.tile([P, F], f32, tag="iota")
            nc.gpsimd.iota(
                iota_t[:], pattern=[[1, F]], base=0, channel_multiplier=0,
                allow_small_or_imprecise_dtypes=True,
            )

            c0n_t = cpool.tile([P, 1], f32, tag="c0n")
            nc.vector.memset(c0n_t[:], -c0)
            negi_t = cpool.tile([P, P], f32, tag="negI")
            nc.sync.dma_start(negi_t[:], negi_d[:, :])
            wcst_t = cpool.tile([4, RPC], bf16, tag="wconst")
            nc.sync.dma_start(wcst_t[:], wcst_d[:, :])

            rows = []
            for rc in range(RC_N):
                r0 = rc * P
                rt = cpool.tile([P, NCONST], f32, tag=f"rows{rc}")
                nc.sync.dma_start(rt[:], rcst_d[r0:r0 + P, :])
                rows.append(rt)
            carry_a1 = [cpool.tile([P, 1], f32, tag=f"ca1_{rc}", name=f"ca1_{rc}")
                        for rc in range(RC_N)]
            carry_d = [cpool.tile([P, 1], f32, tag=f"cd_{rc}", name=f"cd_{rc}")
                       for rc in range(RC_N)]

            for tci in range(TC_N):
                t0 = tci * F
                # R rows for the affine matmuls (bf16 hi/lo split):
                # [ones, ones, iota_hi, iota_lo]
                r_t = wpool.tile([4, F], bf16, tag="R")
                nc.sync.dma_start(r_t[:], rmat_d[:, t0:t0 + F])
                d_tiles = []
                for rc in range(RC_N):
                    r0 = rc * P
                    rt = rows[rc]
                    al1 = rt[:, 0:1]
                    dal = rt[:, 1:2]
                    dbe = rt[:, 2:3]
                    dd = rt[:, 3:4]
                    ab1t = rt[:, 4 + 2 * tci:5 + 2 * tci]
                    ab2t = rt[:, 5 + 2 * tci:6 + 2 * tci]

                    obs_t = wpool.tile([P, F], f32, tag="obs")
                    nc.sync.dma_start(obs_t[:], obs_d[r0:r0 + P, t0:t0 + F])

                    # a1: exclusive scan of obs with init alpha1
                    a1_t = a1pool.tile([P, F + 1], f32, tag="a1")
                    a1_init = al1 if tci == 0 else carry_a1[rc][:]
                    nc.vector.tensor_copy(a1_t[:, 0:1], a1_init)
                    nc.vector.tensor_tensor_scan(
                        a1_t[:, 1:F + 1], obs_t[:], obs_t[:], a1_init,
                        Alu.add, Alu.bypass,
                    )
                    nc.vector.tensor_copy(carry_a1[rc][:], a1_t[:, F:F + 1])
                    a1_v = a1_t[:, 0:F]

                    # PE: b1 = (ab1 + t) - a1 into PSUM
                    b_ps = ppool.tile([P, F], f32, tag="b1")
                    wsl = wcst_t[:, r0:r0 + P]
                    for q in range(F // 512):
                        sl = slice(q * 512, (q + 1) * 512)
                        rsl = r_t[:, q * 512:(q + 1) * 512]
                        nc.tensor.matmul(
                            b_ps[:, sl], wsl, rsl, start=True, stop=False)
                        nc.tensor.matmul(
                            b_ps[:, sl], negi_t[:], a1_v[:, sl],
                            start=False, stop=True)

                    # b1 -> SBUF fp32 (also the b1 output), then outputs
                    b1_t = wpool.tile([P, F], f32, tag="b1sb")
                    nc.scalar.activation(b1_t[:], b_ps[:], Act.Identity)
                    a2_t = wpool.tile([P, F], bf16, tag="a2")
                    nc.scalar.activation(a2_t[:], a1_v, Act.Identity, bias=dal)
                    b2_t = wpool.tile([P, F], bf16, tag="b2")
                    nc.scalar.activation(b2_t[:], b1_t[:], Act.Identity, bias=dbe)
                    nc.sync.dma_start(a1_o[r0:r0 + P, t0:t0 + F], a1_v)
                    nc.sync.dma_start(b1_o[r0:r0 + P, t0:t0 + F], b1_t[:])
                    nc.sync.dma_start(a2_o[r0:r0 + P, t0:t0 + F], a2_t[:])
                    nc.sync.dma_start(b2_o[r0:r0 + P, t0:t0 + F], b2_t[:])

                    # den = obs ? a1 : b1  (in place in SBUF)
                    obs_mask = obs_t[:].bitcast(mybir.dt.uint32)
                    nc.vector.copy_predicated(b1_t[:], obs_mask, a1_v)

                    # seld = obs*dd + dbe (ACT); num = den + seld (GPS)
                    # num2 = (iota + ab1t) * num (DVE stt)
                    # w2t = iota + ab2t (ACT); den2 = den * w2t (GPS)
                    seld_t = wpool.tile([P, F], f32, tag="seld")
                    nc.scalar.activation(
                        seld_t[:], obs_t[:], Act.Identity, bias=dbe, scale=dd)
                    num_t = wpool.tile([P, F], f32, tag="num")
                    nc.gpsimd.tensor_tensor(
                        num_t[:], b1_t[:], seld_t[:], Alu.add)
                    num2_t = wpool.tile([P, F], f32, tag="num2")
                    nc.vector.scalar_tensor_tensor(
                        num2_t[:], iota_t[:], ab1t, num_t[:], Alu.add, Alu.mult)
                    den2_t = wpool.tile([P, F], f32, tag="den2")
                    nc.scalar.activation(
                        den2_t[:], iota_t[:], Act.Identity, bias=ab2t)
                    nc.gpsimd.tensor_tensor(
                        den2_t[:], b1_t[:], den2_t[:], Alu.mult)

                    # logs in place
                    nc.scalar.activation(num2_t[:], num2_t[:], Act.Ln)
                    nc.scalar.activation(den2_t[:], den2_t[:], Act.Ln)

                    # d: state = (lnum2 + state) - lden2, chained
                    d_t = dpool.tile([P, F + 1], f32, tag="d")
                    if tci == 0:
                        nc.vector.memset(d_t[:, 0:1], 0.0)
                        d_init = 0.0
                    else:
                        d_init = carry_d[rc][:]
                        nc.vector.tensor_copy(d_t[:, 0:1], d_init)
                    nc.vector.tensor_tensor_scan(
                        d_t[:, 1:F + 1], num2_t[:], den2_t[:], d_init,
                        Alu.add, Alu.subtract,
                    )
                    nc.vector.tensor_copy(carry_d[rc][:], d_t[:, F:F + 1])
                    d_tiles.append(d_t)

                # post = sigmoid(-(d + c0)), batched so the act-table
                # switches cost 2 loads per t-chunk
                for rc in range(RC_N):
                    r0 = rc * P
                    post_t = wpool.tile([P, F], bf16, tag="post")
                    nc.scalar.activation(
                        post_t[:], d_tiles[rc][:, 0:F], Act.Sigmoid,
                        bias=c0n_t[:], scale=-1.0)
                    nc.sync.dma_start(pm_o[r0:r0 + P, t0:t0 + F], post_t[:])
    nc.finalize()
    return nc


def _pack_inputs(alpha1, beta1, alpha2, beta2):
    """rowconst [B, NCONST]: al1, dal, dbe, dd, then ab2+t0 per chunk.
    wconst [2, B]: row0 = ab1, row1 = ones."""
    a1 = alpha1.astype(np.float32)
    b1 = beta1.astype(np.float32)
    a2 = alpha2.astype(np.float32)
    b2 = beta2.astype(np.float32)
    dal = a2 - a1
    dbe = b2 - b1
    cols = [a1, dal, dbe, dal - dbe]
    ab1 = a1 + b1
    ab2 = a2 + b2
    for tci in range(TC_N):
        cols.append(ab1 + np.float32(tci * F))
        cols.append(ab2 + np.float32(tci * F))
    rowconst = np.ascontiguousarray(np.stack(cols, axis=1), dtype=np.float32)
    import ml_dtypes
    bf = np.dtype(ml_dtypes.bfloat16)
    ab1_hi = (a1 + b1).astype(bf).astype(np.float32)
    ab1_lo = (a1 + b1) - ab1_hi
    ones = np.ones_like(ab1_hi)
    wconst = np.ascontiguousarray(
        np.stack([ab1_hi, ab1_lo, ones, ones], axis=0).astype(bf))
    return rowconst, wconst


def make_in_maps(obs_seq, alpha1, beta1, alpha2, beta2):
    obs_seq = np.ascontiguousarray(obs_seq, dtype=np.float32)
    rowconst, wconst = _pack_inputs(
        np.asarray(alpha1), np.asarray(beta1),
        np.asarray(alpha2), np.asarray(beta2),
    )
    import ml_dtypes
    bf = np.dtype(ml_dtypes.bfloat16)
    negI = np.ascontiguousarray(-np.eye(P, dtype=np.float32))
    g = np.arange(T, dtype=np.float32)
    g_hi = np.floor(g / 32.0) * 32.0
    g_lo = g - g_hi
    onesT = np.ones(T, np.float32)
    rmat = np.ascontiguousarray(
        np.stack([onesT, onesT, g_hi, g_lo], axis=0).astype(bf))
    in_maps = []
    for c in range(NCORES):
        r0 = c * RPC
        in_maps.append({
            "obs": obs_seq[r0:r0 + RPC],
            "rowconst": rowconst[r0:r0 + RPC],
            "wconst": np.ascontiguousarray(wconst[:, r0:r0 + RPC]),
            "negI": negI,
            "rmat": rmat,
        })
    return in_maps


def kernel(obs_seq, alpha1, beta1, alpha2, beta2, mixweight):
    from concourse.bass_utils import run_bass_kernel_spmd

    w = float(np.float32(mixweight))
    c0 = float(np.float32(np.log((1.0 - w) / w)))
    key = c0
    if key not in _PROGRAM_CACHE:
        _PROGRAM_CACHE[key] = _build_program(c0)
    nc = _PROGRAM_CACHE[key]

    in_maps = make_in_maps(obs_seq, alpha1, beta1, alpha2, beta2)
    res = run_bass_kernel_spmd(nc, in_maps, core_ids=list(range(NCORES)))
    out = np.empty((5, B, T), np.float32)
    names = ["a1_out", "b1_out", "a2_out", "b2_out", "post_out"]
    for c in range(NCORES):
        r0 = c * RPC
        for k, name in enumerate(names):
            out[k, r0:r0 + RPC] = np.asarray(res.results[c][name]).astype(np.float32)
    return out
